# revision 14
# baseline (speedup 1.0000x reference)
"""Trainium2 Bass kernel for ContinuousAttentiveStatisticsPooling.

Shape config (hardcoded): B=8, C=256, L=8192, A=128, 8 NeuronCores,
pure data parallel over B (one example per core).

Math restructure (per example, x is [C, L]):
  - Host zeroes x beyond the valid length -> all L-reductions over full L
    equal masked reductions, and W @ x has exact-zero tails.
  - Host prep also folds everything that only depends on the (masked)
    input moments and the weights, the same way it already folds the BN
    affine into Wc' and precomputes 1/total:
      gmean = sum(x)/total ; gstd = sqrt(clip(sum(x^2)/total - gmean^2))
      ch   = Wt2 @ gmean + Wt3 @ gstd + b_tdnn          (relu bias)
      cv   = W2 @ gmean + W3 @ gstd + b_val             (values const)
      pinv = exp(Wc' @ relu(ch))                        (invalid-tail p)
  - Device streams x once:
      vraw   = W1 @ x                   (values, raw: cv added at the end)
      h      = relu(Wt1 @ x + ch)
      p      = exp(Wc' @ h)             (score bias b' dropped: a
                                         per-channel constant cancels in
                                         the softmax over L)
      Z += p ; S1 += p*vraw ; S2 += (p*vraw)*vraw       (accumulators)
  - Invalid tail: x=0 there, so p = pinv exactly;
      Z_valid = Z - n_invalid * pinv, and p*vraw has zero tail.
  - amean = S1/Z + cv ; avar = S2/Z - (S1/Z)^2 ; astd = exp(0.5 ln avar)

Schedule notes (from HW traces):
  - Each issuing engine owns ONE in-order DMA ring (~350GB/s steady after
    a ramp); x chunks are split across the sync and gpsimd rings with the
    small weights/consts ahead of them.
  - Streaming: 1024-wide superblocks. PSUM = v(2cb x 2 banks) + ph(2) +
    s(2) = 8 banks. Per superblock: ACT relu + 2x exp(Z via accum),
    DVE 4x scalar_tensor_tensor (S1/S2 via accum), PE 12x 512-col matmul.
  - exp/ln only (no Sqrt): Square/Ln/Exp/Relu share one ACT table set;
    Sqrt would trigger ~1.3us table reloads.
"""

import sys

if "/opt/trn_rl_repo" not in sys.path:
    sys.path.insert(0, "/opt/trn_rl_repo")

import numpy as np
import ml_dtypes

import concourse.bass as bass
import concourse.mybir as mybir
import concourse.tile as tile
from concourse.tile import add_dep_helper
from concourse.bass_utils import run_bass_kernel_spmd

B, C, L, A = 8, 256, 8192, 128
CB = C // 128          # 2 c-blocks
NSB = 8                # streaming superblocks over L
SB = L // NSB          # 1024
NDMA = 8               # x DMA chunks per c-block
LD = L // NDMA         # 1024
EPS = 1e-12
F32 = mybir.dt.float32
BF16 = mybir.dt.bfloat16
ALU = mybir.AluOpType
ACT = mybir.ActivationFunctionType

_mw_ctr = [0]


def _split_multiwaits(nc):
    """This walrus build supports only ONE sync-wait per instruction.
    Split multi-wait instructions into single-wait NoOps on the same engine
    (same-engine program order preserves semantics exactly)."""
    for f in nc.m.functions:
        for blk in f.blocks:
            insts = blk.instructions
            out = []
            changed = False
            for inst in insts:
                si = inst.sync_info
                if si is not None and len(si.on_wait) > 1:
                    changed = True
                    waits = list(si.on_wait)
                    for w in waits[:-1]:
                        _mw_ctr[0] += 1
                        nop = mybir.InstNoOp(
                            name=f"mwsplit-{_mw_ctr[0]}", ins=[], outs=[]
                        )
                        nop.engine = inst.engine
                        nop.sync_info = mybir.SyncInfo(on_wait=[w], on_update=[])
                        out.append(nop)
                    inst.sync_info = mybir.SyncInfo(
                        on_wait=[waits[-1]], on_update=list(si.on_update)
                    )
                out.append(inst)
            if changed:
                insts[:] = out


def _build_nc():
    nc = bass.Bass()
    x_d = nc.dram_tensor("x", [C, L], BF16, kind="ExternalInput")
    wv1t_d = nc.dram_tensor("wv1t", [128, 2, CB, 128], BF16, kind="ExternalInput")
    wtt_d = nc.dram_tensor("wtt", [128, 2, 128], BF16, kind="ExternalInput")
    wct_d = nc.dram_tensor("wct", [128, CB, 128], BF16, kind="ExternalInput")
    # [ch | cv0 cv1 | pinv0 pinv1 | 1/total | n_invalid] per partition
    cst_d = nc.dram_tensor("cst", [128, 7], F32, kind="ExternalInput")
    out_d = nc.dram_tensor("out", [128, 4], F32, kind="ExternalOutput")

    with tile.TileContext(nc) as tc:
        with (
            tc.tile_pool(name="consts", bufs=1) as cp,
            tc.tile_pool(name="xs", bufs=1) as xp,
            tc.tile_pool(name="hw", bufs=3) as hp,
            tc.tile_pool(name="pw", bufs=4) as pp,
            tc.tile_pool(name="qw", bufs=4) as qp,
            tc.tile_pool(name="q2w", bufs=2) as q2p,
        ):
            # ---- DMAs across THREE in-order rings (sync / gpsimd /
            # scalar): chunk 0 rides the otherwise-idle scalar ring so
            # streaming starts early; weights lead their ring. ----
            zz = cp.tile([128, 1], F32, tag="zz", name="zz")
            nc.vector.memset(zz, 0)
            zzo = cp.tile([128, 1], F32, tag="zzo", name="zzo")
            # dummy activation: forces the ACT table load at t~0
            nc.scalar.activation(out=zzo, in_=zz, func=ACT.Relu)
            # PE warm-up: dummy matmuls raise the PE p-state before the
            # first real matmul arrives
            wuL = cp.tile([128, 128], BF16, tag="wuL", name="wuL")
            nc.vector.memset(wuL, 0)
            wuR = cp.tile([128, 512], BF16, tag="wuR", name="wuR")
            nc.vector.memset(wuR, 0)

            wtt = cp.tile([128, 2, 128], BF16, tag="wtt", name="wtt")
            nc.scalar.dma_start(out=wtt, in_=wtt_d[:, :, :])
            wct = cp.tile([128, CB, 128], BF16, tag="wct", name="wct")
            nc.scalar.dma_start(out=wct, in_=wct_d[:, :, :])
            cst = cp.tile([128, 7], F32, tag="cst", name="cst")
            nc.scalar.dma_start(out=cst, in_=cst_d[:, :])
            wv1t = cp.tile([128, 2, CB, 128], BF16, tag="wv1t", name="wv1t")
            nc.gpsimd.dma_start(out=wv1t, in_=wv1t_d[:, :, :, :])

            xs = [[xp.tile([128, LD], BF16, tag=f"x{cb}_{j}", name=f"x{cb}_{j}")
                   for j in range(NDMA)] for cb in range(CB)]
            nc.sync.dma_start(out=xs[0][0], in_=x_d[0:128, 0:LD])
            nc.sync.dma_start(out=xs[1][0], in_=x_d[128:256, 0:LD])
            for j in range(1, NDMA):
                for cb in range(CB):
                    eng = nc.sync if (j + cb) % 2 == 0 else nc.gpsimd
                    eng.dma_start(
                        out=xs[cb][j],
                        in_=x_d[cb * 128 : (cb + 1) * 128, j * LD : (j + 1) * LD],
                    )

            ch = cst[:, 0:1]

            # streaming accumulators (2D tiles: STT accum_out must be 2D)
            Zp = [cp.tile([128, NSB], F32, tag=f"Zp{cb}", name=f"Zp{cb}") for cb in range(CB)]
            S1p = [cp.tile([128, NSB], F32, tag=f"S1p{cb}", name=f"S1p{cb}") for cb in range(CB)]
            S2p = [cp.tile([128, NSB], F32, tag=f"S2p{cb}", name=f"S2p{cb}") for cb in range(CB)]

            with (
                tc.tile_pool(name="psv", bufs=2, space="PSUM") as ps_v,
                tc.tile_pool(name="pss", bufs=2, space="PSUM") as ps_s,
            ):
                def emit_ph(k):
                    ph = ps_s.tile([128, SB], F32, tag="s", name="ph")
                    for half in range(2):
                        hsl = slice(half * 512, (half + 1) * 512)
                        nc.tensor.matmul(ph[:, hsl], lhsT=wtt[:, 0, :], rhs=xs[0][k][:, hsl], start=True, stop=False)
                        nc.tensor.matmul(ph[:, hsl], lhsT=wtt[:, 1, :], rhs=xs[1][k][:, hsl], start=False, stop=True)
                    return ph

                def emit_v(k, cb):
                    vps = ps_v.tile([128, SB], F32, tag="v", name="v")
                    for half in range(2):
                        hsl = slice(half * 512, (half + 1) * 512)
                        nc.tensor.matmul(vps[:, hsl], lhsT=wv1t[:, 0, cb, :], rhs=xs[0][k][:, hsl], start=True, stop=False)
                        nc.tensor.matmul(vps[:, hsl], lhsT=wv1t[:, 1, cb, :], rhs=xs[1][k][:, hsl], start=False, stop=True)
                    return vps

                wups = ps_v.tile([128, SB], F32, tag="v", name="wups")
                for _ in range(18):
                    nc.tensor.matmul(wups[:, 0:512], lhsT=wuL, rhs=wuR,
                                     start=True, stop=True)
                ph_next = emit_ph(0)
                v_next = {cb: emit_v(0, cb) for cb in range(CB)}

                for k in range(NSB):
                    ph = ph_next
                    vk = v_next
                    h = hp.tile([128, SB], BF16, tag="h", name="h")
                    nc.scalar.activation(out=h, in_=ph, func=ACT.Relu, bias=ch)
                    for cb in range(CB):
                        sps = ps_s.tile([128, SB], F32, tag="s", name="s")
                        for half in range(2):
                            hsl = slice(half * 512, (half + 1) * 512)
                            nc.tensor.matmul(sps[:, hsl], lhsT=wct[:, cb, :],
                                             rhs=h[:, hsl], start=True, stop=True)
                        if cb == 0 and k + 1 < NSB:
                            ph_next = emit_ph(k + 1)
                        p = pp.tile([128, SB], BF16, tag="p", name="p")
                        nc.scalar.activation(
                            out=p, in_=sps, func=ACT.Exp,
                            accum_out=Zp[cb][:, k : k + 1],
                        )
                        q = qp.tile([128, SB], BF16, tag="q", name="q")
                        nc.vector.scalar_tensor_tensor(
                            out=q, in0=p, scalar=0.0, in1=vk[cb],
                            op0=ALU.bypass, op1=ALU.mult,
                            accum_out=S1p[cb][:, k : k + 1],
                        )
                        q2 = q2p.tile([128, SB], BF16, tag="q2", name="q2")
                        nc.vector.scalar_tensor_tensor(
                            out=q2, in0=q, scalar=0.0, in1=vk[cb],
                            op0=ALU.bypass, op1=ALU.mult,
                            accum_out=S2p[cb][:, k : k + 1],
                        )
                    if k + 1 < NSB:
                        v_next = {cb: emit_v(k + 1, cb) for cb in range(CB)}

            # ---- finalize (c-blocks batched in [128,2] ops) ----
            zs = cp.tile([128, 6], F32, tag="zs", name="zs")
            for i, t in enumerate([Zp[0], Zp[1], S1p[0], S1p[1], S2p[0], S2p[1]]):
                nc.vector.tensor_reduce(out=zs[:, i : i + 1], in_=t,
                                        axis=mybir.AxisListType.X, op=ALU.add)
            corr = cp.tile([128, CB], F32, tag="corr", name="corr")
            nc.vector.tensor_scalar_mul(out=corr, in0=cst[:, 3:5], scalar1=cst[:, 6:7])
            Zv = cp.tile([128, CB], F32, tag="Zv", name="Zv")
            nc.vector.tensor_sub(out=Zv, in0=zs[:, 0:2], in1=corr)
            rz = cp.tile([128, CB], F32, tag="rz", name="rz")
            nc.vector.reciprocal(out=rz, in_=Zv)
            m1 = cp.tile([128, CB], F32, tag="m1", name="m1")
            nc.vector.tensor_mul(out=m1, in0=zs[:, 2:4], in1=rz)
            staging = cp.tile([128, 4], F32, tag="staging", name="staging")
            nc.vector.tensor_add(out=staging[:, 0:CB], in0=m1, in1=cst[:, 1:3])
            t1 = cp.tile([128, CB], F32, tag="t1", name="t1")
            nc.vector.tensor_mul(out=t1, in0=zs[:, 4:6], in1=rz)
            m1sq = cp.tile([128, CB], F32, tag="m1sq", name="m1sq")
            nc.vector.tensor_mul(out=m1sq, in0=m1, in1=m1)
            avar = cp.tile([128, CB], F32, tag="avar", name="avar")
            nc.vector.tensor_sub(out=avar, in0=t1, in1=m1sq)
            nc.vector.tensor_scalar_max(out=avar, in0=avar, scalar1=EPS)
            lnv = cp.tile([128, CB], F32, tag="lnv", name="lnv")
            nc.scalar.activation(out=lnv, in_=avar, func=ACT.Ln)
            nc.scalar.activation(out=staging[:, CB : 2 * CB], in_=lnv, func=ACT.Exp, scale=0.5)
            nc.scalar.dma_start(out=out_d[:, :], in_=staging)

    _split_multiwaits(nc)
    return nc


_NC_CACHE = None


def _get_nc():
    global _NC_CACHE
    if _NC_CACHE is None:
        _NC_CACHE = _build_nc()
    return _NC_CACHE


def _prep_inputs(x, lengths, w_val, b_val, w_tdnn, b_tdnn, bn_gamma, bn_beta,
                 w_conv, b_conv):
    x = np.asarray(x, dtype=np.float32)
    lengths = np.asarray(lengths, dtype=np.float32)
    w_val = np.asarray(w_val, dtype=np.float32)
    b_val = np.asarray(b_val, dtype=np.float32)
    w_tdnn = np.asarray(w_tdnn, dtype=np.float32)
    b_tdnn = np.asarray(b_tdnn, dtype=np.float32)
    bn_gamma = np.asarray(bn_gamma, dtype=np.float32)
    bn_beta = np.asarray(bn_beta, dtype=np.float32)
    w_conv = np.asarray(w_conv, dtype=np.float32)
    b_conv = np.asarray(b_conv, dtype=np.float32)

    mask = (np.arange(L, dtype=np.float32)[None, :] < (lengths * L)[:, None])
    total = mask.sum(axis=1).astype(np.float32)            # [B]
    xm = (x * mask[:, None, :].astype(np.float32)).astype(ml_dtypes.bfloat16)
    xf = xm.astype(np.float32)

    # masked global moments (from the bf16-rounded x the device also sees)
    gmean = xf.sum(axis=2) / total[:, None]                                  # [B, C]
    gsq = (xf * xf).sum(axis=2) / total[:, None]
    gstd = np.sqrt(np.clip(gsq - gmean * gmean, EPS, None))                  # [B, C]

    def pack_lhsT(w, kblocks, cblocks, dt=None):
        # w: [K, M] (contraction-major) -> [128, kblocks, cblocks, 128]
        Ktot, Mtot = w.shape
        assert Ktot == kblocks * 128 and Mtot == cblocks * 128
        r = np.ascontiguousarray(
            w.reshape(kblocks, 128, cblocks, 128).transpose(1, 0, 2, 3)
        )
        return r.astype(dt) if dt is not None else r

    W1T = w_val[:, :C].T                                   # [f, c]
    wv1t = pack_lhsT(W1T, 2, CB, ml_dtypes.bfloat16)
    WtT = w_tdnn[:, :C].T                                  # [f, a]
    wtt = pack_lhsT(WtT, 2, 1, ml_dtypes.bfloat16).reshape(128, 2, 128)
    WcT = (w_conv * bn_gamma[None, :]).T                   # [a, c] (BN gamma folded)
    wct = pack_lhsT(WcT, 1, CB, ml_dtypes.bfloat16).reshape(128, CB, 128)
    # score bias b' = b_conv + w_conv @ bn_beta is constant per channel
    # -> cancels in the softmax; not needed anywhere.

    shared = {"wv1t": wv1t, "wtt": wtt, "wct": wct}
    in_maps = []
    for b in range(B):
        m = dict(shared)
        m["x"] = np.ascontiguousarray(xm[b])
        # per-example folded consts
        gcat = np.concatenate([gmean[b], gstd[b]])                           # [2C]
        ch = w_tdnn[:, C:] @ gcat + b_tdnn                                   # [A]
        cv = w_val[:, C:] @ gcat + b_val                                     # [C]
        hinv = np.maximum(ch, 0.0).astype(ml_dtypes.bfloat16).astype(np.float32)
        sinv = WcT.astype(ml_dtypes.bfloat16).astype(np.float32).T @ hinv    # [C]
        pinv = np.exp(sinv)
        cstm = np.empty((128, 7), dtype=np.float32)
        cstm[:, 0] = ch
        cstm[:, 1:3] = cv.reshape(CB, 128).T
        cstm[:, 3:5] = pinv.reshape(CB, 128).T
        cstm[:, 5] = 1.0 / total[b]
        cstm[:, 6] = L - total[b]
        m["cst"] = np.ascontiguousarray(cstm)
        in_maps.append(m)
    return in_maps


def kernel(**inputs) -> np.ndarray:
    in_maps = _prep_inputs(**inputs)
    nc = _get_nc()
    res = run_bass_kernel_spmd(nc, in_maps, core_ids=list(range(B)))
    # device output is [128, 4] with columns [amean0, amean1, astd0, astd1]
    out = np.empty((B, 2 * C, 1), dtype=np.float32)
    for b in range(B):
        o = res.results[b]["out"]
        out[b, :, 0] = o.T.reshape(2 * C)
    return out


# revision 15
# speedup vs baseline: 1.0148x; 1.0148x over previous
"""Trainium2 Bass kernel for ContinuousAttentiveStatisticsPooling.

Shape config (hardcoded): B=8, C=256, L=8192, A=128, 8 NeuronCores,
pure data parallel over B (one example per core).

Math restructure (per example, x is [C, L]):
  - Host zeroes x beyond the valid length -> all L-reductions over full L
    equal masked reductions, and W @ x has exact-zero tails.
  - Host prep also folds everything that only depends on the (masked)
    input moments and the weights, the same way it already folds the BN
    affine into Wc' and precomputes 1/total:
      gmean = sum(x)/total ; gstd = sqrt(clip(sum(x^2)/total - gmean^2))
      ch   = Wt2 @ gmean + Wt3 @ gstd + b_tdnn          (relu bias)
      cv   = W2 @ gmean + W3 @ gstd + b_val             (values const)
      pinv = exp(Wc' @ relu(ch))                        (invalid-tail p)
  - Device streams x once:
      vraw   = W1 @ x                   (values, raw: cv added at the end)
      h      = relu(Wt1 @ x + ch)
      p      = exp(Wc' @ h)             (score bias b' dropped: a
                                         per-channel constant cancels in
                                         the softmax over L)
      Z += p ; S1 += p*vraw ; S2 += (p*vraw)*vraw       (accumulators)
  - Invalid tail: x=0 there, so p = pinv exactly;
      Z_valid = Z - n_invalid * pinv, and p*vraw has zero tail.
  - amean = S1/Z + cv ; avar = S2/Z - (S1/Z)^2 ; astd = exp(0.5 ln avar)

Schedule notes (from HW traces):
  - Each issuing engine owns ONE in-order DMA ring (~350GB/s steady after
    a ramp); x chunks are split across the sync and gpsimd rings with the
    small weights/consts ahead of them.
  - Streaming: 1024-wide superblocks. PSUM = v(2cb x 2 banks) + ph(2) +
    s(2) = 8 banks. Per superblock: ACT relu + 2x exp(Z via accum),
    DVE 4x scalar_tensor_tensor (S1/S2 via accum), PE 12x 512-col matmul.
  - exp/ln only (no Sqrt): Square/Ln/Exp/Relu share one ACT table set;
    Sqrt would trigger ~1.3us table reloads.
"""

import sys

if "/opt/trn_rl_repo" not in sys.path:
    sys.path.insert(0, "/opt/trn_rl_repo")

import numpy as np
import ml_dtypes

import concourse.bass as bass
import concourse.mybir as mybir
import concourse.tile as tile
from concourse.tile import add_dep_helper
from concourse.bass_utils import run_bass_kernel_spmd

B, C, L, A = 8, 256, 8192, 128
CB = C // 128          # 2 c-blocks
NSB = 8                # streaming superblocks over L
SB = L // NSB          # 1024
NDMA = 8               # x DMA chunks per c-block
LD = L // NDMA         # 1024
EPS = 1e-12
F32 = mybir.dt.float32
BF16 = mybir.dt.bfloat16
ALU = mybir.AluOpType
ACT = mybir.ActivationFunctionType

_mw_ctr = [0]


def _split_multiwaits(nc):
    """This walrus build supports only ONE sync-wait per instruction.
    Split multi-wait instructions into single-wait NoOps on the same engine
    (same-engine program order preserves semantics exactly)."""
    for f in nc.m.functions:
        for blk in f.blocks:
            insts = blk.instructions
            out = []
            changed = False
            for inst in insts:
                si = inst.sync_info
                if si is not None and len(si.on_wait) > 1:
                    changed = True
                    waits = list(si.on_wait)
                    for w in waits[:-1]:
                        _mw_ctr[0] += 1
                        nop = mybir.InstNoOp(
                            name=f"mwsplit-{_mw_ctr[0]}", ins=[], outs=[]
                        )
                        nop.engine = inst.engine
                        nop.sync_info = mybir.SyncInfo(on_wait=[w], on_update=[])
                        out.append(nop)
                    inst.sync_info = mybir.SyncInfo(
                        on_wait=[waits[-1]], on_update=list(si.on_update)
                    )
                out.append(inst)
            if changed:
                insts[:] = out


def _build_nc():
    nc = bass.Bass()
    x_d = nc.dram_tensor("x", [C, L], BF16, kind="ExternalInput")
    wv1t_d = nc.dram_tensor("wv1t", [128, 2, CB, 128], BF16, kind="ExternalInput")
    wtt_d = nc.dram_tensor("wtt", [128, 2, 128], BF16, kind="ExternalInput")
    wct_d = nc.dram_tensor("wct", [128, CB, 128], BF16, kind="ExternalInput")
    # [ch | cv0 cv1 | pinv0 pinv1 | 1/total | n_invalid] per partition
    cst_d = nc.dram_tensor("cst", [128, 7], F32, kind="ExternalInput")
    out_d = nc.dram_tensor("out", [128, 4], F32, kind="ExternalOutput")

    with tile.TileContext(nc) as tc:
        with (
            tc.tile_pool(name="consts", bufs=1) as cp,
            tc.tile_pool(name="xs", bufs=1) as xp,
            tc.tile_pool(name="hw", bufs=3) as hp,
            tc.tile_pool(name="pw", bufs=4) as pp,
            tc.tile_pool(name="qw", bufs=4) as qp,
            tc.tile_pool(name="q2w", bufs=2) as q2p,
        ):
            # ---- DMAs across THREE in-order rings (sync / gpsimd /
            # scalar): chunk 0 rides the otherwise-idle scalar ring so
            # streaming starts early; weights lead their ring. ----
            zz = cp.tile([128, 1], F32, tag="zz", name="zz")
            nc.vector.memset(zz, 0)
            zzo = cp.tile([128, 1], F32, tag="zzo", name="zzo")
            # dummy activation: forces the ACT table load at t~0
            nc.scalar.activation(out=zzo, in_=zz, func=ACT.Relu)
            # PE warm-up: dummy matmuls raise the PE p-state before the
            # first real matmul arrives
            wuL = cp.tile([128, 128], BF16, tag="wuL", name="wuL")
            nc.vector.memset(wuL, 0)
            wuR = cp.tile([128, 512], BF16, tag="wuR", name="wuR")
            nc.vector.memset(wuR, 0)

            wtt = cp.tile([128, 2, 128], BF16, tag="wtt", name="wtt")
            nc.sync.dma_start(out=wtt, in_=wtt_d[:, :, :])
            wct = cp.tile([128, CB, 128], BF16, tag="wct", name="wct")
            nc.sync.dma_start(out=wct, in_=wct_d[:, :, :])
            cst = cp.tile([128, 7], F32, tag="cst", name="cst")
            nc.gpsimd.dma_start(out=cst, in_=cst_d[:, :])
            wv1t = cp.tile([128, 2, CB, 128], BF16, tag="wv1t", name="wv1t")
            nc.gpsimd.dma_start(out=wv1t, in_=wv1t_d[:, :, :, :])

            xs = [[xp.tile([128, LD], BF16, tag=f"x{cb}_{j}", name=f"x{cb}_{j}")
                   for j in range(NDMA)] for cb in range(CB)]
            nc.scalar.dma_start(out=xs[0][0], in_=x_d[0:128, 0:LD])
            nc.scalar.dma_start(out=xs[1][0], in_=x_d[128:256, 0:LD])
            for j in range(1, NDMA):
                for cb in range(CB):
                    eng = nc.sync if (j + cb) % 2 == 0 else nc.gpsimd
                    eng.dma_start(
                        out=xs[cb][j],
                        in_=x_d[cb * 128 : (cb + 1) * 128, j * LD : (j + 1) * LD],
                    )

            ch = cst[:, 0:1]

            # streaming accumulators (2D tiles: STT accum_out must be 2D)
            Zp = [cp.tile([128, NSB], F32, tag=f"Zp{cb}", name=f"Zp{cb}") for cb in range(CB)]
            S1p = [cp.tile([128, NSB], F32, tag=f"S1p{cb}", name=f"S1p{cb}") for cb in range(CB)]
            S2p = [cp.tile([128, NSB], F32, tag=f"S2p{cb}", name=f"S2p{cb}") for cb in range(CB)]

            with (
                tc.tile_pool(name="psv", bufs=2, space="PSUM") as ps_v,
                tc.tile_pool(name="pss", bufs=2, space="PSUM") as ps_s,
            ):
                def emit_ph(k):
                    ph = ps_s.tile([128, SB], F32, tag="s", name="ph")
                    for half in range(2):
                        hsl = slice(half * 512, (half + 1) * 512)
                        nc.tensor.matmul(ph[:, hsl], lhsT=wtt[:, 0, :], rhs=xs[0][k][:, hsl], start=True, stop=False)
                        nc.tensor.matmul(ph[:, hsl], lhsT=wtt[:, 1, :], rhs=xs[1][k][:, hsl], start=False, stop=True)
                    return ph

                def emit_v(k, cb):
                    vps = ps_v.tile([128, SB], F32, tag="v", name="v")
                    for half in range(2):
                        hsl = slice(half * 512, (half + 1) * 512)
                        nc.tensor.matmul(vps[:, hsl], lhsT=wv1t[:, 0, cb, :], rhs=xs[0][k][:, hsl], start=True, stop=False)
                        nc.tensor.matmul(vps[:, hsl], lhsT=wv1t[:, 1, cb, :], rhs=xs[1][k][:, hsl], start=False, stop=True)
                    return vps

                wups = ps_v.tile([128, SB], F32, tag="v", name="wups")
                for _ in range(18):
                    nc.tensor.matmul(wups[:, 0:512], lhsT=wuL, rhs=wuR,
                                     start=True, stop=True)
                ph_next = emit_ph(0)
                v_next = {cb: emit_v(0, cb) for cb in range(CB)}

                for k in range(NSB):
                    ph = ph_next
                    vk = v_next
                    h = hp.tile([128, SB], BF16, tag="h", name="h")
                    nc.scalar.activation(out=h, in_=ph, func=ACT.Relu, bias=ch)
                    for cb in range(CB):
                        sps = ps_s.tile([128, SB], F32, tag="s", name="s")
                        for half in range(2):
                            hsl = slice(half * 512, (half + 1) * 512)
                            nc.tensor.matmul(sps[:, hsl], lhsT=wct[:, cb, :],
                                             rhs=h[:, hsl], start=True, stop=True)
                        if cb == 0 and k + 1 < NSB:
                            ph_next = emit_ph(k + 1)
                        p = pp.tile([128, SB], BF16, tag="p", name="p")
                        nc.scalar.activation(
                            out=p, in_=sps, func=ACT.Exp,
                            accum_out=Zp[cb][:, k : k + 1],
                        )
                        q = qp.tile([128, SB], BF16, tag="q", name="q")
                        nc.vector.scalar_tensor_tensor(
                            out=q, in0=p, scalar=0.0, in1=vk[cb],
                            op0=ALU.bypass, op1=ALU.mult,
                            accum_out=S1p[cb][:, k : k + 1],
                        )
                        q2 = q2p.tile([128, SB], BF16, tag="q2", name="q2")
                        nc.vector.scalar_tensor_tensor(
                            out=q2, in0=q, scalar=0.0, in1=vk[cb],
                            op0=ALU.bypass, op1=ALU.mult,
                            accum_out=S2p[cb][:, k : k + 1],
                        )
                    if k + 1 < NSB:
                        v_next = {cb: emit_v(k + 1, cb) for cb in range(CB)}

            # ---- finalize (c-blocks batched in [128,2] ops) ----
            zs = cp.tile([128, 6], F32, tag="zs", name="zs")
            for i, t in enumerate([Zp[0], Zp[1], S1p[0], S1p[1], S2p[0], S2p[1]]):
                nc.vector.tensor_reduce(out=zs[:, i : i + 1], in_=t,
                                        axis=mybir.AxisListType.X, op=ALU.add)
            corr = cp.tile([128, CB], F32, tag="corr", name="corr")
            nc.vector.tensor_scalar_mul(out=corr, in0=cst[:, 3:5], scalar1=cst[:, 6:7])
            Zv = cp.tile([128, CB], F32, tag="Zv", name="Zv")
            nc.vector.tensor_sub(out=Zv, in0=zs[:, 0:2], in1=corr)
            rz = cp.tile([128, CB], F32, tag="rz", name="rz")
            nc.vector.reciprocal(out=rz, in_=Zv)
            m1 = cp.tile([128, CB], F32, tag="m1", name="m1")
            nc.vector.tensor_mul(out=m1, in0=zs[:, 2:4], in1=rz)
            staging = cp.tile([128, 4], F32, tag="staging", name="staging")
            nc.vector.tensor_add(out=staging[:, 0:CB], in0=m1, in1=cst[:, 1:3])
            t1 = cp.tile([128, CB], F32, tag="t1", name="t1")
            nc.vector.tensor_mul(out=t1, in0=zs[:, 4:6], in1=rz)
            m1sq = cp.tile([128, CB], F32, tag="m1sq", name="m1sq")
            nc.vector.tensor_mul(out=m1sq, in0=m1, in1=m1)
            avar = cp.tile([128, CB], F32, tag="avar", name="avar")
            nc.vector.tensor_sub(out=avar, in0=t1, in1=m1sq)
            nc.vector.tensor_scalar_max(out=avar, in0=avar, scalar1=EPS)
            lnv = cp.tile([128, CB], F32, tag="lnv", name="lnv")
            nc.scalar.activation(out=lnv, in_=avar, func=ACT.Ln)
            nc.scalar.activation(out=staging[:, CB : 2 * CB], in_=lnv, func=ACT.Exp, scale=0.5)
            nc.scalar.dma_start(out=out_d[:, :], in_=staging)

    _split_multiwaits(nc)
    return nc


_NC_CACHE = None


def _get_nc():
    global _NC_CACHE
    if _NC_CACHE is None:
        _NC_CACHE = _build_nc()
    return _NC_CACHE


def _prep_inputs(x, lengths, w_val, b_val, w_tdnn, b_tdnn, bn_gamma, bn_beta,
                 w_conv, b_conv):
    x = np.asarray(x, dtype=np.float32)
    lengths = np.asarray(lengths, dtype=np.float32)
    w_val = np.asarray(w_val, dtype=np.float32)
    b_val = np.asarray(b_val, dtype=np.float32)
    w_tdnn = np.asarray(w_tdnn, dtype=np.float32)
    b_tdnn = np.asarray(b_tdnn, dtype=np.float32)
    bn_gamma = np.asarray(bn_gamma, dtype=np.float32)
    bn_beta = np.asarray(bn_beta, dtype=np.float32)
    w_conv = np.asarray(w_conv, dtype=np.float32)
    b_conv = np.asarray(b_conv, dtype=np.float32)

    mask = (np.arange(L, dtype=np.float32)[None, :] < (lengths * L)[:, None])
    total = mask.sum(axis=1).astype(np.float32)            # [B]
    xm = (x * mask[:, None, :].astype(np.float32)).astype(ml_dtypes.bfloat16)
    xf = xm.astype(np.float32)

    # masked global moments (from the bf16-rounded x the device also sees)
    gmean = xf.sum(axis=2) / total[:, None]                                  # [B, C]
    gsq = (xf * xf).sum(axis=2) / total[:, None]
    gstd = np.sqrt(np.clip(gsq - gmean * gmean, EPS, None))                  # [B, C]

    def pack_lhsT(w, kblocks, cblocks, dt=None):
        # w: [K, M] (contraction-major) -> [128, kblocks, cblocks, 128]
        Ktot, Mtot = w.shape
        assert Ktot == kblocks * 128 and Mtot == cblocks * 128
        r = np.ascontiguousarray(
            w.reshape(kblocks, 128, cblocks, 128).transpose(1, 0, 2, 3)
        )
        return r.astype(dt) if dt is not None else r

    W1T = w_val[:, :C].T                                   # [f, c]
    wv1t = pack_lhsT(W1T, 2, CB, ml_dtypes.bfloat16)
    WtT = w_tdnn[:, :C].T                                  # [f, a]
    wtt = pack_lhsT(WtT, 2, 1, ml_dtypes.bfloat16).reshape(128, 2, 128)
    WcT = (w_conv * bn_gamma[None, :]).T                   # [a, c] (BN gamma folded)
    wct = pack_lhsT(WcT, 1, CB, ml_dtypes.bfloat16).reshape(128, CB, 128)
    # score bias b' = b_conv + w_conv @ bn_beta is constant per channel
    # -> cancels in the softmax; not needed anywhere.

    shared = {"wv1t": wv1t, "wtt": wtt, "wct": wct}
    in_maps = []
    for b in range(B):
        m = dict(shared)
        m["x"] = np.ascontiguousarray(xm[b])
        # per-example folded consts
        gcat = np.concatenate([gmean[b], gstd[b]])                           # [2C]
        ch = w_tdnn[:, C:] @ gcat + b_tdnn                                   # [A]
        cv = w_val[:, C:] @ gcat + b_val                                     # [C]
        hinv = np.maximum(ch, 0.0).astype(ml_dtypes.bfloat16).astype(np.float32)
        sinv = WcT.astype(ml_dtypes.bfloat16).astype(np.float32).T @ hinv    # [C]
        pinv = np.exp(sinv)
        cstm = np.empty((128, 7), dtype=np.float32)
        cstm[:, 0] = ch
        cstm[:, 1:3] = cv.reshape(CB, 128).T
        cstm[:, 3:5] = pinv.reshape(CB, 128).T
        cstm[:, 5] = 1.0 / total[b]
        cstm[:, 6] = L - total[b]
        m["cst"] = np.ascontiguousarray(cstm)
        in_maps.append(m)
    return in_maps


def kernel(**inputs) -> np.ndarray:
    in_maps = _prep_inputs(**inputs)
    nc = _get_nc()
    res = run_bass_kernel_spmd(nc, in_maps, core_ids=list(range(B)))
    # device output is [128, 4] with columns [amean0, amean1, astd0, astd1]
    out = np.empty((B, 2 * C, 1), dtype=np.float32)
    for b in range(B):
        o = res.results[b]["out"]
        out[b, :, 0] = o.T.reshape(2 * C)
    return out


# revision 16
# speedup vs baseline: 1.0310x; 1.0160x over previous
"""Trainium2 Bass kernel for ContinuousAttentiveStatisticsPooling.

Shape config (hardcoded): B=8, C=256, L=8192, A=128, 8 NeuronCores,
pure data parallel over B (one example per core).

Math restructure (per example, x is [C, L]):
  - Host zeroes x beyond the valid length -> all L-reductions over full L
    equal masked reductions, and W @ x has exact-zero tails.
  - Host prep also folds everything that only depends on the (masked)
    input moments and the weights, the same way it already folds the BN
    affine into Wc' and precomputes 1/total:
      gmean = sum(x)/total ; gstd = sqrt(clip(sum(x^2)/total - gmean^2))
      ch   = Wt2 @ gmean + Wt3 @ gstd + b_tdnn          (relu bias)
      cv   = W2 @ gmean + W3 @ gstd + b_val             (values const)
      pinv = exp(Wc' @ relu(ch))                        (invalid-tail p)
  - Device streams x once:
      vraw   = W1 @ x                   (values, raw: cv added at the end)
      h      = relu(Wt1 @ x + ch)
      p      = exp(Wc' @ h)             (score bias b' dropped: a
                                         per-channel constant cancels in
                                         the softmax over L)
      Z += p ; S1 += p*vraw ; S2 += (p*vraw)*vraw       (accumulators)
  - Invalid tail: x=0 there, so p = pinv exactly;
      Z_valid = Z - n_invalid * pinv, and p*vraw has zero tail.
  - amean = S1/Z + cv ; avar = S2/Z - (S1/Z)^2 ; astd = exp(0.5 ln avar)

Schedule notes (from HW traces):
  - Each issuing engine owns ONE in-order DMA ring (~350GB/s steady after
    a ramp); x chunks are split across the sync and gpsimd rings with the
    small weights/consts ahead of them.
  - Streaming: 1024-wide superblocks. PSUM = v(2cb x 2 banks) + ph(2) +
    s(2) = 8 banks. Per superblock: ACT relu + 2x exp(Z via accum),
    DVE 4x scalar_tensor_tensor (S1/S2 via accum), PE 12x 512-col matmul.
  - exp/ln only (no Sqrt): Square/Ln/Exp/Relu share one ACT table set;
    Sqrt would trigger ~1.3us table reloads.
"""

import sys

if "/opt/trn_rl_repo" not in sys.path:
    sys.path.insert(0, "/opt/trn_rl_repo")

import numpy as np
import ml_dtypes

import concourse.bass as bass
import concourse.mybir as mybir
import concourse.tile as tile
from concourse.tile import add_dep_helper
from concourse.bass_utils import run_bass_kernel_spmd

B, C, L, A = 8, 256, 8192, 128
CB = C // 128          # 2 c-blocks
NSB = 8                # streaming superblocks over L
SB = L // NSB          # 1024
NDMA = 8               # x DMA chunks per c-block
LD = L // NDMA         # 1024
EPS = 1e-12
F32 = mybir.dt.float32
BF16 = mybir.dt.bfloat16
ALU = mybir.AluOpType
ACT = mybir.ActivationFunctionType

_mw_ctr = [0]


def _split_multiwaits(nc):
    """This walrus build supports only ONE sync-wait per instruction.
    Split multi-wait instructions into single-wait NoOps on the same engine
    (same-engine program order preserves semantics exactly)."""
    for f in nc.m.functions:
        for blk in f.blocks:
            insts = blk.instructions
            out = []
            changed = False
            for inst in insts:
                si = inst.sync_info
                if si is not None and len(si.on_wait) > 1:
                    changed = True
                    waits = list(si.on_wait)
                    for w in waits[:-1]:
                        _mw_ctr[0] += 1
                        nop = mybir.InstNoOp(
                            name=f"mwsplit-{_mw_ctr[0]}", ins=[], outs=[]
                        )
                        nop.engine = inst.engine
                        nop.sync_info = mybir.SyncInfo(on_wait=[w], on_update=[])
                        out.append(nop)
                    inst.sync_info = mybir.SyncInfo(
                        on_wait=[waits[-1]], on_update=list(si.on_update)
                    )
                out.append(inst)
            if changed:
                insts[:] = out


def _build_nc():
    nc = bass.Bass()
    x_d = nc.dram_tensor("x", [C, L], BF16, kind="ExternalInput")
    wv1t_d = nc.dram_tensor("wv1t", [128, 2, CB, 128], BF16, kind="ExternalInput")
    wtt_d = nc.dram_tensor("wtt", [128, 2, 128], BF16, kind="ExternalInput")
    wct_d = nc.dram_tensor("wct", [128, CB, 128], BF16, kind="ExternalInput")
    # [ch | cv0 cv1 | pinv0 pinv1 | 1/total | n_invalid] per partition
    cst_d = nc.dram_tensor("cst", [128, 7], F32, kind="ExternalInput")
    out_d = nc.dram_tensor("out", [128, 4], F32, kind="ExternalOutput")

    with tile.TileContext(nc) as tc:
        with (
            tc.tile_pool(name="consts", bufs=1) as cp,
            tc.tile_pool(name="xs", bufs=1) as xp,
            tc.tile_pool(name="hw", bufs=3) as hp,
            tc.tile_pool(name="pw", bufs=4) as pp,
            tc.tile_pool(name="qw", bufs=4) as qp,
            tc.tile_pool(name="q2w", bufs=2) as q2p,
        ):
            # ---- DMAs across THREE in-order rings (sync / gpsimd /
            # scalar): chunk 0 rides the otherwise-idle scalar ring so
            # streaming starts early; weights lead their ring. ----
            zz = cp.tile([128, 1], F32, tag="zz", name="zz")
            nc.vector.memset(zz, 0)
            zzo = cp.tile([128, 1], F32, tag="zzo", name="zzo")
            # dummy activation: forces the ACT table load at t~0
            nc.scalar.activation(out=zzo, in_=zz, func=ACT.Relu)
            # PE warm-up: dummy matmuls raise the PE p-state before the
            # first real matmul arrives
            wuL = cp.tile([128, 128], BF16, tag="wuL", name="wuL")
            nc.vector.memset(wuL, 0)
            wuR = cp.tile([128, 512], BF16, tag="wuR", name="wuR")
            nc.vector.memset(wuR, 0)

            x10 = xp.tile([128, LD], BF16, tag="x1_0", name="x1_0")
            nc.sync.dma_start(out=x10, in_=x_d[128:256, 0:LD])
            wtt = cp.tile([128, 2, 128], BF16, tag="wtt", name="wtt")
            nc.sync.dma_start(out=wtt, in_=wtt_d[:, :, :])
            wct = cp.tile([128, CB, 128], BF16, tag="wct", name="wct")
            nc.sync.dma_start(out=wct, in_=wct_d[:, :, :])
            cst = cp.tile([128, 7], F32, tag="cst", name="cst")
            nc.gpsimd.dma_start(out=cst, in_=cst_d[:, :])
            wv1t = cp.tile([128, 2, CB, 128], BF16, tag="wv1t", name="wv1t")
            nc.gpsimd.dma_start(out=wv1t, in_=wv1t_d[:, :, :, :])

            xs = [[xp.tile([128, LD], BF16, tag=f"x{cb}_{j}", name=f"x{cb}_{j}")
                   for j in range(NDMA)] for cb in range(CB)]
            nc.scalar.dma_start(out=xs[0][0], in_=x_d[0:128, 0:LD])
            xs[1][0] = x10
            for j in range(1, NDMA):
                for cb in range(CB):
                    eng = nc.sync if (j + cb) % 2 == 0 else nc.gpsimd
                    eng.dma_start(
                        out=xs[cb][j],
                        in_=x_d[cb * 128 : (cb + 1) * 128, j * LD : (j + 1) * LD],
                    )

            ch = cst[:, 0:1]

            # streaming accumulators (2D tiles: STT accum_out must be 2D)
            Zp = [cp.tile([128, NSB], F32, tag=f"Zp{cb}", name=f"Zp{cb}") for cb in range(CB)]
            S1p = [cp.tile([128, NSB], F32, tag=f"S1p{cb}", name=f"S1p{cb}") for cb in range(CB)]
            S2p = [cp.tile([128, NSB], F32, tag=f"S2p{cb}", name=f"S2p{cb}") for cb in range(CB)]

            with (
                tc.tile_pool(name="psv", bufs=2, space="PSUM") as ps_v,
                tc.tile_pool(name="pss", bufs=2, space="PSUM") as ps_s,
            ):
                def emit_ph(k):
                    ph = ps_s.tile([128, SB], F32, tag="s", name="ph")
                    for half in range(2):
                        hsl = slice(half * 512, (half + 1) * 512)
                        nc.tensor.matmul(ph[:, hsl], lhsT=wtt[:, 0, :], rhs=xs[0][k][:, hsl], start=True, stop=False)
                        nc.tensor.matmul(ph[:, hsl], lhsT=wtt[:, 1, :], rhs=xs[1][k][:, hsl], start=False, stop=True)
                    return ph

                def emit_v(k, cb):
                    vps = ps_v.tile([128, SB], F32, tag="v", name="v")
                    for half in range(2):
                        hsl = slice(half * 512, (half + 1) * 512)
                        nc.tensor.matmul(vps[:, hsl], lhsT=wv1t[:, 0, cb, :], rhs=xs[0][k][:, hsl], start=True, stop=False)
                        nc.tensor.matmul(vps[:, hsl], lhsT=wv1t[:, 1, cb, :], rhs=xs[1][k][:, hsl], start=False, stop=True)
                    return vps

                wups = ps_v.tile([128, SB], F32, tag="v", name="wups")
                for _ in range(12):
                    nc.tensor.matmul(wups[:, 0:512], lhsT=wuL, rhs=wuR,
                                     start=True, stop=True)
                ph_next = emit_ph(0)
                v_next = {cb: emit_v(0, cb) for cb in range(CB)}

                for k in range(NSB):
                    ph = ph_next
                    vk = v_next
                    h = hp.tile([128, SB], BF16, tag="h", name="h")
                    nc.scalar.activation(out=h, in_=ph, func=ACT.Relu, bias=ch)
                    for cb in range(CB):
                        sps = ps_s.tile([128, SB], F32, tag="s", name="s")
                        for half in range(2):
                            hsl = slice(half * 512, (half + 1) * 512)
                            nc.tensor.matmul(sps[:, hsl], lhsT=wct[:, cb, :],
                                             rhs=h[:, hsl], start=True, stop=True)
                        if cb == 0 and k + 1 < NSB:
                            ph_next = emit_ph(k + 1)
                        p = pp.tile([128, SB], BF16, tag="p", name="p")
                        nc.scalar.activation(
                            out=p, in_=sps, func=ACT.Exp,
                            accum_out=Zp[cb][:, k : k + 1],
                        )
                        q = qp.tile([128, SB], BF16, tag="q", name="q")
                        nc.vector.scalar_tensor_tensor(
                            out=q, in0=p, scalar=0.0, in1=vk[cb],
                            op0=ALU.bypass, op1=ALU.mult,
                            accum_out=S1p[cb][:, k : k + 1],
                        )
                        q2 = q2p.tile([128, SB], BF16, tag="q2", name="q2")
                        nc.vector.scalar_tensor_tensor(
                            out=q2, in0=q, scalar=0.0, in1=vk[cb],
                            op0=ALU.bypass, op1=ALU.mult,
                            accum_out=S2p[cb][:, k : k + 1],
                        )
                    if k + 1 < NSB:
                        v_next = {cb: emit_v(k + 1, cb) for cb in range(CB)}

            # ---- finalize (c-blocks batched in [128,2] ops) ----
            zs = cp.tile([128, 6], F32, tag="zs", name="zs")
            for i, t in enumerate([Zp[0], Zp[1], S1p[0], S1p[1], S2p[0], S2p[1]]):
                nc.vector.tensor_reduce(out=zs[:, i : i + 1], in_=t,
                                        axis=mybir.AxisListType.X, op=ALU.add)
            corr = cp.tile([128, CB], F32, tag="corr", name="corr")
            nc.vector.tensor_scalar_mul(out=corr, in0=cst[:, 3:5], scalar1=cst[:, 6:7])
            Zv = cp.tile([128, CB], F32, tag="Zv", name="Zv")
            nc.vector.tensor_sub(out=Zv, in0=zs[:, 0:2], in1=corr)
            rz = cp.tile([128, CB], F32, tag="rz", name="rz")
            nc.vector.reciprocal(out=rz, in_=Zv)
            m1 = cp.tile([128, CB], F32, tag="m1", name="m1")
            nc.vector.tensor_mul(out=m1, in0=zs[:, 2:4], in1=rz)
            staging = cp.tile([128, 4], F32, tag="staging", name="staging")
            nc.vector.tensor_add(out=staging[:, 0:CB], in0=m1, in1=cst[:, 1:3])
            t1 = cp.tile([128, CB], F32, tag="t1", name="t1")
            nc.vector.tensor_mul(out=t1, in0=zs[:, 4:6], in1=rz)
            m1sq = cp.tile([128, CB], F32, tag="m1sq", name="m1sq")
            nc.vector.tensor_mul(out=m1sq, in0=m1, in1=m1)
            avar = cp.tile([128, CB], F32, tag="avar", name="avar")
            nc.vector.tensor_sub(out=avar, in0=t1, in1=m1sq)
            nc.vector.tensor_scalar_max(out=avar, in0=avar, scalar1=EPS)
            lnv = cp.tile([128, CB], F32, tag="lnv", name="lnv")
            nc.scalar.activation(out=lnv, in_=avar, func=ACT.Ln)
            nc.scalar.activation(out=staging[:, CB : 2 * CB], in_=lnv, func=ACT.Exp, scale=0.5)
            nc.scalar.dma_start(out=out_d[:, :], in_=staging)

    _split_multiwaits(nc)
    return nc


_NC_CACHE = None


def _get_nc():
    global _NC_CACHE
    if _NC_CACHE is None:
        _NC_CACHE = _build_nc()
    return _NC_CACHE


def _prep_inputs(x, lengths, w_val, b_val, w_tdnn, b_tdnn, bn_gamma, bn_beta,
                 w_conv, b_conv):
    x = np.asarray(x, dtype=np.float32)
    lengths = np.asarray(lengths, dtype=np.float32)
    w_val = np.asarray(w_val, dtype=np.float32)
    b_val = np.asarray(b_val, dtype=np.float32)
    w_tdnn = np.asarray(w_tdnn, dtype=np.float32)
    b_tdnn = np.asarray(b_tdnn, dtype=np.float32)
    bn_gamma = np.asarray(bn_gamma, dtype=np.float32)
    bn_beta = np.asarray(bn_beta, dtype=np.float32)
    w_conv = np.asarray(w_conv, dtype=np.float32)
    b_conv = np.asarray(b_conv, dtype=np.float32)

    mask = (np.arange(L, dtype=np.float32)[None, :] < (lengths * L)[:, None])
    total = mask.sum(axis=1).astype(np.float32)            # [B]
    xm = (x * mask[:, None, :].astype(np.float32)).astype(ml_dtypes.bfloat16)
    xf = xm.astype(np.float32)

    # masked global moments (from the bf16-rounded x the device also sees)
    gmean = xf.sum(axis=2) / total[:, None]                                  # [B, C]
    gsq = (xf * xf).sum(axis=2) / total[:, None]
    gstd = np.sqrt(np.clip(gsq - gmean * gmean, EPS, None))                  # [B, C]

    def pack_lhsT(w, kblocks, cblocks, dt=None):
        # w: [K, M] (contraction-major) -> [128, kblocks, cblocks, 128]
        Ktot, Mtot = w.shape
        assert Ktot == kblocks * 128 and Mtot == cblocks * 128
        r = np.ascontiguousarray(
            w.reshape(kblocks, 128, cblocks, 128).transpose(1, 0, 2, 3)
        )
        return r.astype(dt) if dt is not None else r

    W1T = w_val[:, :C].T                                   # [f, c]
    wv1t = pack_lhsT(W1T, 2, CB, ml_dtypes.bfloat16)
    WtT = w_tdnn[:, :C].T                                  # [f, a]
    wtt = pack_lhsT(WtT, 2, 1, ml_dtypes.bfloat16).reshape(128, 2, 128)
    WcT = (w_conv * bn_gamma[None, :]).T                   # [a, c] (BN gamma folded)
    wct = pack_lhsT(WcT, 1, CB, ml_dtypes.bfloat16).reshape(128, CB, 128)
    # score bias b' = b_conv + w_conv @ bn_beta is constant per channel
    # -> cancels in the softmax; not needed anywhere.

    shared = {"wv1t": wv1t, "wtt": wtt, "wct": wct}
    in_maps = []
    for b in range(B):
        m = dict(shared)
        m["x"] = np.ascontiguousarray(xm[b])
        # per-example folded consts
        gcat = np.concatenate([gmean[b], gstd[b]])                           # [2C]
        ch = w_tdnn[:, C:] @ gcat + b_tdnn                                   # [A]
        cv = w_val[:, C:] @ gcat + b_val                                     # [C]
        hinv = np.maximum(ch, 0.0).astype(ml_dtypes.bfloat16).astype(np.float32)
        sinv = WcT.astype(ml_dtypes.bfloat16).astype(np.float32).T @ hinv    # [C]
        pinv = np.exp(sinv)
        cstm = np.empty((128, 7), dtype=np.float32)
        cstm[:, 0] = ch
        cstm[:, 1:3] = cv.reshape(CB, 128).T
        cstm[:, 3:5] = pinv.reshape(CB, 128).T
        cstm[:, 5] = 1.0 / total[b]
        cstm[:, 6] = L - total[b]
        m["cst"] = np.ascontiguousarray(cstm)
        in_maps.append(m)
    return in_maps


def kernel(**inputs) -> np.ndarray:
    in_maps = _prep_inputs(**inputs)
    nc = _get_nc()
    res = run_bass_kernel_spmd(nc, in_maps, core_ids=list(range(B)))
    # device output is [128, 4] with columns [amean0, amean1, astd0, astd1]
    out = np.empty((B, 2 * C, 1), dtype=np.float32)
    for b in range(B):
        o = res.results[b]["out"]
        out[b, :, 0] = o.T.reshape(2 * C)
    return out


# revision 17
# speedup vs baseline: 1.1184x; 1.0847x over previous
"""Trainium2 Bass kernel for ContinuousAttentiveStatisticsPooling.

Shape config (hardcoded): B=8, C=256, L=8192, A=128, 8 NeuronCores,
pure data parallel over B (one example per core).

Math restructure (per example, x is [C, L]):
  - Host zeroes x beyond the valid length -> all L-reductions over full L
    equal masked reductions, and W @ x has exact-zero tails.
  - Host prep also folds everything that only depends on the (masked)
    input moments and the weights, the same way it already folds the BN
    affine into Wc' and precomputes 1/total:
      gmean = sum(x)/total ; gstd = sqrt(clip(sum(x^2)/total - gmean^2))
      ch   = Wt2 @ gmean + Wt3 @ gstd + b_tdnn          (relu bias)
      cv   = W2 @ gmean + W3 @ gstd + b_val             (values const)
      pinv = exp(Wc' @ relu(ch))                        (invalid-tail p)
  - Device streams x once:
      vraw   = W1 @ x                   (values, raw: cv added at the end)
      h      = relu(Wt1 @ x + ch)
      p      = exp(Wc' @ h)             (score bias b' dropped: a
                                         per-channel constant cancels in
                                         the softmax over L)
      Z += p ; S1 += p*vraw ; S2 += (p*vraw)*vraw       (accumulators)
  - Invalid tail: x=0 there, so p = pinv exactly;
      Z_valid = Z - n_invalid * pinv, and p*vraw has zero tail.
  - amean = S1/Z + cv ; avar = S2/Z - (S1/Z)^2 ; astd = exp(0.5 ln avar)

Schedule notes (from HW traces):
  - Each issuing engine owns ONE in-order DMA ring (~350GB/s steady after
    a ramp); x chunks are split across the sync and gpsimd rings with the
    small weights/consts ahead of them.
  - Streaming: 1024-wide superblocks. PSUM = v(2cb x 2 banks) + ph(2) +
    s(2) = 8 banks. Per superblock: ACT relu + 2x exp(Z via accum),
    DVE 4x scalar_tensor_tensor (S1/S2 via accum), PE 12x 512-col matmul.
  - exp/ln only (no Sqrt): Square/Ln/Exp/Relu share one ACT table set;
    Sqrt would trigger ~1.3us table reloads.
"""

import sys

if "/opt/trn_rl_repo" not in sys.path:
    sys.path.insert(0, "/opt/trn_rl_repo")

import numpy as np
import ml_dtypes

import concourse.bass as bass
import concourse.mybir as mybir
import concourse.tile as tile
from concourse.tile import add_dep_helper
from concourse.bass_utils import run_bass_kernel_spmd

B, C, L, A = 8, 256, 8192, 128
CB = C // 128          # 2 c-blocks
NSB = 8                # streaming superblocks over L
SB = L // NSB          # 1024
NDMA = 8               # x DMA chunks per c-block
LD = L // NDMA         # 1024
EPS = 1e-12
F32 = mybir.dt.float32
BF16 = mybir.dt.bfloat16
ALU = mybir.AluOpType
ACT = mybir.ActivationFunctionType

_mw_ctr = [0]


def _split_multiwaits(nc):
    """This walrus build supports only ONE sync-wait per instruction.
    Split multi-wait instructions into single-wait NoOps on the same engine
    (same-engine program order preserves semantics exactly)."""
    for f in nc.m.functions:
        for blk in f.blocks:
            insts = blk.instructions
            out = []
            changed = False
            for inst in insts:
                si = inst.sync_info
                if si is not None and len(si.on_wait) > 1:
                    changed = True
                    waits = list(si.on_wait)
                    for w in waits[:-1]:
                        _mw_ctr[0] += 1
                        nop = mybir.InstNoOp(
                            name=f"mwsplit-{_mw_ctr[0]}", ins=[], outs=[]
                        )
                        nop.engine = inst.engine
                        nop.sync_info = mybir.SyncInfo(on_wait=[w], on_update=[])
                        out.append(nop)
                    inst.sync_info = mybir.SyncInfo(
                        on_wait=[waits[-1]], on_update=list(si.on_update)
                    )
                out.append(inst)
            if changed:
                insts[:] = out


def _build_nc(n_sb):
    nc = bass.Bass()
    x_d = nc.dram_tensor("x", [C, L], BF16, kind="ExternalInput")
    wv1t_d = nc.dram_tensor("wv1t", [128, 2, CB, 128], BF16, kind="ExternalInput")
    wtt_d = nc.dram_tensor("wtt", [128, 2, 128], BF16, kind="ExternalInput")
    wct_d = nc.dram_tensor("wct", [128, CB, 128], BF16, kind="ExternalInput")
    # [ch | cv0 cv1 | pinv0 pinv1 | 1/total | n_invalid] per partition
    cst_d = nc.dram_tensor("cst", [128, 7], F32, kind="ExternalInput")
    out_d = nc.dram_tensor("out", [128, 4], F32, kind="ExternalOutput")

    with tile.TileContext(nc) as tc:
        with (
            tc.tile_pool(name="consts", bufs=1) as cp,
            tc.tile_pool(name="xs", bufs=1) as xp,
            tc.tile_pool(name="hw", bufs=3) as hp,
            tc.tile_pool(name="pw", bufs=4) as pp,
            tc.tile_pool(name="qw", bufs=4) as qp,
            tc.tile_pool(name="q2w", bufs=2) as q2p,
        ):
            # ---- DMAs across THREE in-order rings (sync / gpsimd /
            # scalar): chunk 0 rides the otherwise-idle scalar ring so
            # streaming starts early; weights lead their ring. ----
            zz = cp.tile([128, 1], F32, tag="zz", name="zz")
            nc.vector.memset(zz, 0)
            zzo = cp.tile([128, 1], F32, tag="zzo", name="zzo")
            # dummy activation: forces the ACT table load at t~0
            nc.scalar.activation(out=zzo, in_=zz, func=ACT.Relu)
            # PE warm-up: dummy matmuls raise the PE p-state before the
            # first real matmul arrives
            wuL = cp.tile([128, 128], BF16, tag="wuL", name="wuL")
            nc.vector.memset(wuL, 0)
            wuR = cp.tile([128, 512], BF16, tag="wuR", name="wuR")
            nc.vector.memset(wuR, 0)

            x10 = xp.tile([128, LD], BF16, tag="x1_0", name="x1_0")
            nc.sync.dma_start(out=x10, in_=x_d[128:256, 0:LD])
            wtt = cp.tile([128, 2, 128], BF16, tag="wtt", name="wtt")
            nc.sync.dma_start(out=wtt, in_=wtt_d[:, :, :])
            wct = cp.tile([128, CB, 128], BF16, tag="wct", name="wct")
            nc.sync.dma_start(out=wct, in_=wct_d[:, :, :])
            cst = cp.tile([128, 7], F32, tag="cst", name="cst")
            nc.gpsimd.dma_start(out=cst, in_=cst_d[:, :])
            wv1t = cp.tile([128, 2, CB, 128], BF16, tag="wv1t", name="wv1t")
            nc.gpsimd.dma_start(out=wv1t, in_=wv1t_d[:, :, :, :])

            xs = [[xp.tile([128, LD], BF16, tag=f"x{cb}_{j}", name=f"x{cb}_{j}")
                   for j in range(n_sb)] for cb in range(CB)]
            nc.scalar.dma_start(out=xs[0][0], in_=x_d[0:128, 0:LD])
            xs[1][0] = x10
            for j in range(1, n_sb):
                for cb in range(CB):
                    eng = nc.sync if (j + cb) % 2 == 0 else nc.gpsimd
                    eng.dma_start(
                        out=xs[cb][j],
                        in_=x_d[cb * 128 : (cb + 1) * 128, j * LD : (j + 1) * LD],
                    )

            ch = cst[:, 0:1]

            # streaming accumulators (2D tiles: STT accum_out must be 2D)
            Zp = [cp.tile([128, n_sb], F32, tag=f"Zp{cb}", name=f"Zp{cb}") for cb in range(CB)]
            S1p = [cp.tile([128, n_sb], F32, tag=f"S1p{cb}", name=f"S1p{cb}") for cb in range(CB)]
            S2p = [cp.tile([128, n_sb], F32, tag=f"S2p{cb}", name=f"S2p{cb}") for cb in range(CB)]

            with (
                tc.tile_pool(name="psv", bufs=2, space="PSUM") as ps_v,
                tc.tile_pool(name="pss", bufs=2, space="PSUM") as ps_s,
            ):
                def emit_ph(k):
                    ph = ps_s.tile([128, SB], F32, tag="s", name="ph")
                    for half in range(2):
                        hsl = slice(half * 512, (half + 1) * 512)
                        nc.tensor.matmul(ph[:, hsl], lhsT=wtt[:, 0, :], rhs=xs[0][k][:, hsl], start=True, stop=False)
                        nc.tensor.matmul(ph[:, hsl], lhsT=wtt[:, 1, :], rhs=xs[1][k][:, hsl], start=False, stop=True)
                    return ph

                def emit_v(k, cb):
                    vps = ps_v.tile([128, SB], F32, tag="v", name="v")
                    for half in range(2):
                        hsl = slice(half * 512, (half + 1) * 512)
                        nc.tensor.matmul(vps[:, hsl], lhsT=wv1t[:, 0, cb, :], rhs=xs[0][k][:, hsl], start=True, stop=False)
                        nc.tensor.matmul(vps[:, hsl], lhsT=wv1t[:, 1, cb, :], rhs=xs[1][k][:, hsl], start=False, stop=True)
                    return vps

                wups = ps_v.tile([128, SB], F32, tag="v", name="wups")
                for _ in range(12):
                    nc.tensor.matmul(wups[:, 0:512], lhsT=wuL, rhs=wuR,
                                     start=True, stop=True)
                ph_next = emit_ph(0)
                v_next = {cb: emit_v(0, cb) for cb in range(CB)}

                for k in range(n_sb):
                    ph = ph_next
                    vk = v_next
                    h = hp.tile([128, SB], BF16, tag="h", name="h")
                    nc.scalar.activation(out=h, in_=ph, func=ACT.Relu, bias=ch)
                    for cb in range(CB):
                        sps = ps_s.tile([128, SB], F32, tag="s", name="s")
                        for half in range(2):
                            hsl = slice(half * 512, (half + 1) * 512)
                            nc.tensor.matmul(sps[:, hsl], lhsT=wct[:, cb, :],
                                             rhs=h[:, hsl], start=True, stop=True)
                        if cb == 0 and k + 1 < n_sb:
                            ph_next = emit_ph(k + 1)
                        p = pp.tile([128, SB], BF16, tag="p", name="p")
                        nc.scalar.activation(
                            out=p, in_=sps, func=ACT.Exp,
                            accum_out=Zp[cb][:, k : k + 1],
                        )
                        q = qp.tile([128, SB], BF16, tag="q", name="q")
                        nc.vector.scalar_tensor_tensor(
                            out=q, in0=p, scalar=0.0, in1=vk[cb],
                            op0=ALU.bypass, op1=ALU.mult,
                            accum_out=S1p[cb][:, k : k + 1],
                        )
                        q2 = q2p.tile([128, SB], BF16, tag="q2", name="q2")
                        nc.vector.scalar_tensor_tensor(
                            out=q2, in0=q, scalar=0.0, in1=vk[cb],
                            op0=ALU.bypass, op1=ALU.mult,
                            accum_out=S2p[cb][:, k : k + 1],
                        )
                    if k + 1 < n_sb:
                        v_next = {cb: emit_v(k + 1, cb) for cb in range(CB)}

            # ---- finalize (c-blocks batched in [128,2] ops) ----
            zs = cp.tile([128, 6], F32, tag="zs", name="zs")
            for i, t in enumerate([Zp[0], Zp[1], S1p[0], S1p[1], S2p[0], S2p[1]]):
                nc.vector.tensor_reduce(out=zs[:, i : i + 1], in_=t,
                                        axis=mybir.AxisListType.X, op=ALU.add)
            corr = cp.tile([128, CB], F32, tag="corr", name="corr")
            nc.vector.tensor_scalar_mul(out=corr, in0=cst[:, 3:5], scalar1=cst[:, 6:7])
            Zv = cp.tile([128, CB], F32, tag="Zv", name="Zv")
            nc.vector.tensor_sub(out=Zv, in0=zs[:, 0:2], in1=corr)
            rz = cp.tile([128, CB], F32, tag="rz", name="rz")
            nc.vector.reciprocal(out=rz, in_=Zv)
            m1 = cp.tile([128, CB], F32, tag="m1", name="m1")
            nc.vector.tensor_mul(out=m1, in0=zs[:, 2:4], in1=rz)
            staging = cp.tile([128, 4], F32, tag="staging", name="staging")
            nc.vector.tensor_add(out=staging[:, 0:CB], in0=m1, in1=cst[:, 1:3])
            t1 = cp.tile([128, CB], F32, tag="t1", name="t1")
            nc.vector.tensor_mul(out=t1, in0=zs[:, 4:6], in1=rz)
            m1sq = cp.tile([128, CB], F32, tag="m1sq", name="m1sq")
            nc.vector.tensor_mul(out=m1sq, in0=m1, in1=m1)
            avar = cp.tile([128, CB], F32, tag="avar", name="avar")
            nc.vector.tensor_sub(out=avar, in0=t1, in1=m1sq)
            nc.vector.tensor_scalar_max(out=avar, in0=avar, scalar1=EPS)
            lnv = cp.tile([128, CB], F32, tag="lnv", name="lnv")
            nc.scalar.activation(out=lnv, in_=avar, func=ACT.Ln)
            nc.scalar.activation(out=staging[:, CB : 2 * CB], in_=lnv, func=ACT.Exp, scale=0.5)
            nc.scalar.dma_start(out=out_d[:, :], in_=staging)

    _split_multiwaits(nc)
    return nc


_NC_CACHE = {}


def _get_nc(n_sb):
    if n_sb not in _NC_CACHE:
        _NC_CACHE[n_sb] = _build_nc(n_sb)
    return _NC_CACHE[n_sb]


def _prep_inputs(x, lengths, w_val, b_val, w_tdnn, b_tdnn, bn_gamma, bn_beta,
                 w_conv, b_conv):
    x = np.asarray(x, dtype=np.float32)
    lengths = np.asarray(lengths, dtype=np.float32)
    w_val = np.asarray(w_val, dtype=np.float32)
    b_val = np.asarray(b_val, dtype=np.float32)
    w_tdnn = np.asarray(w_tdnn, dtype=np.float32)
    b_tdnn = np.asarray(b_tdnn, dtype=np.float32)
    bn_gamma = np.asarray(bn_gamma, dtype=np.float32)
    bn_beta = np.asarray(bn_beta, dtype=np.float32)
    w_conv = np.asarray(w_conv, dtype=np.float32)
    b_conv = np.asarray(b_conv, dtype=np.float32)

    mask = (np.arange(L, dtype=np.float32)[None, :] < (lengths * L)[:, None])
    total = mask.sum(axis=1).astype(np.float32)            # [B]
    # superblocks beyond every core's valid range contribute nothing to
    # S1/S2 (x=0 there) and a closed-form pinv term to Z: skip them.
    n_sb = int(min(NSB, max(1, np.ceil(total.max() / SB))))
    xm = (x * mask[:, None, :].astype(np.float32)).astype(ml_dtypes.bfloat16)
    xf = xm.astype(np.float32)

    # masked global moments (from the bf16-rounded x the device also sees)
    gmean = xf.sum(axis=2) / total[:, None]                                  # [B, C]
    gsq = (xf * xf).sum(axis=2) / total[:, None]
    gstd = np.sqrt(np.clip(gsq - gmean * gmean, EPS, None))                  # [B, C]

    def pack_lhsT(w, kblocks, cblocks, dt=None):
        # w: [K, M] (contraction-major) -> [128, kblocks, cblocks, 128]
        Ktot, Mtot = w.shape
        assert Ktot == kblocks * 128 and Mtot == cblocks * 128
        r = np.ascontiguousarray(
            w.reshape(kblocks, 128, cblocks, 128).transpose(1, 0, 2, 3)
        )
        return r.astype(dt) if dt is not None else r

    W1T = w_val[:, :C].T                                   # [f, c]
    wv1t = pack_lhsT(W1T, 2, CB, ml_dtypes.bfloat16)
    WtT = w_tdnn[:, :C].T                                  # [f, a]
    wtt = pack_lhsT(WtT, 2, 1, ml_dtypes.bfloat16).reshape(128, 2, 128)
    WcT = (w_conv * bn_gamma[None, :]).T                   # [a, c] (BN gamma folded)
    wct = pack_lhsT(WcT, 1, CB, ml_dtypes.bfloat16).reshape(128, CB, 128)
    # score bias b' = b_conv + w_conv @ bn_beta is constant per channel
    # -> cancels in the softmax; not needed anywhere.

    shared = {"wv1t": wv1t, "wtt": wtt, "wct": wct}
    in_maps = []
    for b in range(B):
        m = dict(shared)
        m["x"] = np.ascontiguousarray(xm[b])
        # per-example folded consts
        gcat = np.concatenate([gmean[b], gstd[b]])                           # [2C]
        ch = w_tdnn[:, C:] @ gcat + b_tdnn                                   # [A]
        cv = w_val[:, C:] @ gcat + b_val                                     # [C]
        hinv = np.maximum(ch, 0.0).astype(ml_dtypes.bfloat16).astype(np.float32)
        sinv = WcT.astype(ml_dtypes.bfloat16).astype(np.float32).T @ hinv    # [C]
        pinv = np.exp(sinv)
        cstm = np.empty((128, 7), dtype=np.float32)
        cstm[:, 0] = ch
        cstm[:, 1:3] = cv.reshape(CB, 128).T
        cstm[:, 3:5] = pinv.reshape(CB, 128).T
        cstm[:, 5] = 1.0 / total[b]
        cstm[:, 6] = n_sb * SB - total[b]
        m["cst"] = np.ascontiguousarray(cstm)
        in_maps.append(m)
    return in_maps, n_sb


def kernel(**inputs) -> np.ndarray:
    in_maps, n_sb = _prep_inputs(**inputs)
    nc = _get_nc(n_sb)
    res = run_bass_kernel_spmd(nc, in_maps, core_ids=list(range(B)))
    # device output is [128, 4] with columns [amean0, amean1, astd0, astd1]
    out = np.empty((B, 2 * C, 1), dtype=np.float32)
    for b in range(B):
        o = res.results[b]["out"]
        out[b, :, 0] = o.T.reshape(2 * C)
    return out
